# revision 1
# baseline (speedup 1.0000x reference)
"""DiT block with 2-token frame attention on 8 trn2 NeuronCores.

Sharding: data-parallel over B=8 frames (one frame per core, SPMD, no
collectives).  Per-frame restructure:

  softmax over 2 kv tokens == sigmoid  =>  y = v1 + p (x) (v0-v1)   (rank-1)
  p_n = sigmoid(SCALE*(rstd_n*(x_n.f - mu_n*sum(f)) + g))
      f = (1+sc_msa) o (w2 @ (k0-k1)),  g = sh_msa.(w2@(k0-k1)) + b2.(k0-k1)
  torch scramble swapaxes(1,2).reshape == reinterpret flat(y^T) as [N, C]
  MLP LN+modulate folded into the fc1 matmul:
      rhs rows  = y2^T o G[c] o rstd2_n   (G = (1+g_msa)(1+sc_mlp))
      extras (K=3 matmul): [u; sh@W; b_fc1]^T @ [-rstd2*mu2; 1; 1]
      gelu applied during PSUM eviction on ACT
  final: out^T = (1+g_msa) o y2^T + g_mlp o mlp^T + g_mlp o b_fc2   (all
      per-partition ops in feature-major layout; host transposes back)

All matmuls bf16 (weights host-cast), accumulation/statistics fp32.
"""

import sys

if "/opt/trn_rl_repo" not in sys.path:
    sys.path.insert(0, "/opt/trn_rl_repo")

from contextlib import ExitStack

import numpy as np
import ml_dtypes

import concourse.bass as bass
import concourse.bacc as bacc
import concourse.mybir as mybir
from concourse import tile
from concourse.bass_utils import run_bass_kernel_spmd
from concourse.masks import make_identity

F32 = mybir.dt.float32
F32R = mybir.dt.float32r
BF16 = mybir.dt.float16  # 16-bit storage: fp16 (10-bit mantissa)
AF = mybir.ActivationFunctionType
ALU = mybir.AluOpType
AX = mybir.AxisListType

C = 1152
KO = C // 128            # 9
H = 4608
HO = H // 128            # 36
M6 = 6 * C               # 6912
SCALE = float((C // 16) ** -0.5)
EPS = 1e-6
NCHUNK = 512


def _nsplits(total, step):
    out, o = [], 0
    while o < total:
        out.append((o, min(step, total - o)))
        o += step
    return out


TAPS = []          # list of (name, shape, dtype) filled by _frame when DEBUG_TAPS


def build_program(ntok=4096, repeat=1):
    nc = bacc.Bacc("TRN2", target_bir_lowering=False, debug=False, num_devices=8)

    io = {
        "xT": nc.dram_tensor("xT", [C, ntok], F32R, kind="ExternalInput"),
        "x0T": nc.dram_tensor("x0T", [128, KO, 2], F32R, kind="ExternalInput"),
        "c_col": nc.dram_tensor("c_col", [128, KO], F32, kind="ExternalInput"),
        "w_ada": nc.dram_tensor("w_ada", [C, M6], BF16, kind="ExternalInput"),
        "b_ada": nc.dram_tensor("b_ada", [1, M6], F32, kind="ExternalInput"),
        "w1": nc.dram_tensor("w1", [C, 2 * C], F32R, kind="ExternalInput"),
        "b1_2": nc.dram_tensor("b1_2", [2, 2 * C], F32, kind="ExternalInput"),
        "w2Ta": nc.dram_tensor("w2Ta", [C, C + 2], F32R, kind="ExternalInput"),
        "wf1": nc.dram_tensor("wf1", [C, H], BF16, kind="ExternalInput"),
        "bf1b": nc.dram_tensor("bf1b", [1, H], BF16, kind="ExternalInput"),
        "wf2": nc.dram_tensor("wf2", [H, C], BF16, kind="ExternalInput"),
        "bf2c": nc.dram_tensor("bf2c", [128, KO], F32, kind="ExternalInput"),
        "outT": nc.dram_tensor("outT", [C, ntok], F32, kind="ExternalOutput"),
    }
    import os
    if os.environ.get("DEBUG_TAPS"):
        io["_taps"] = {}
    with tile.TileContext(nc) as tc:
        for rep in range(repeat):
            _frame(nc, tc, ntok, io, rep)
    nc.compile()
    return nc


def _frame(nc, tc, ntok, io, rep):
    def tap(name, ap):
        if "_taps" not in io or rep != 0:
            return
        import os
        sel = os.environ.get("DEBUG_TAPS", "")
        if sel != "1" and name not in sel.split(","):
            return
        t = nc.dram_tensor(f"tap_{name}", list(ap.shape), ap.dtype,
                           kind="ExternalOutput")
        io["_taps"][name] = t
        nc.sync.dma_start(t.ap(), ap)

    NB = ntok // NCHUNK

    xT_r = io["xT"].ap().rearrange("(ko p) n -> p ko n", p=128)
    wada_r = io["w_ada"].ap().rearrange("(ko p) n -> p ko n", p=128)
    w1_r = io["w1"].ap().rearrange("(ko p) n -> p ko n", p=128)
    w2_r = io["w2Ta"].ap().rearrange("(ko p) n -> p ko n", p=128)
    wf1_r = io["wf1"].ap().rearrange("(ko p) n -> p ko n", p=128)
    wf2_r = io["wf2"].ap().rearrange("(ho p) n -> p ho n", p=128)

    persist = ExitStack()
    pp = persist.enter_context(tc.tile_pool(name=f"persist{rep}", bufs=1))
    dram_p = persist.enter_context(
        tc.tile_pool(name=f"dram{rep}", bufs=1, space="DRAM"))
    flat_t = dram_p.tile([C * ntok], BF16, name=f"flats{rep}")
    flat_za = flat_t.rearrange("(a b) -> a b", b=ntok)    # [C, ntok] Z rows
    flat_tok = flat_t.rearrange("(n c) -> n c", c=C)      # [ntok, C] y2 rows

    ident = pp.tile([128, 128], F32, name=f"ident{rep}")
    make_identity(nc, ident[:])

    y2T = pp.tile([128, KO, ntok], BF16, name=f"y2T{rep}")          # 72 KB
    cols = pp.tile([128, 7 * KO], F32, name=f"cols{rep}")
    kvT = pp.tile([128, 2 * KO, 2], F32, name=f"kvT{rep}")
    smsa_row = pp.tile([1, C], F32, name=f"smsa{rep}")
    shmsa_row = pp.tile([1, C], F32, name=f"shmsa{rep}")
    gsc_s = pp.tile([1, 1], F32, name=f"gscs{rep}")
    silu_bf = pp.tile([128, KO], BF16, name=f"silu{rep}")
    ones_cb = pp.tile([128, KO], BF16, name=f"onescb{rep}")
    ones_cf = pp.tile([128, KO], F32R, name=f"onescf{rep}")
    ones_rb = pp.tile([1, 128], BF16, name=f"onesrb{rep}")
    ones_rr = pp.tile([1, 128], F32R, name=f"onesrr{rep}")
    ones_rf = pp.tile([1, 128], F32, name=f"onesrf{rep}")
    gb2_c = pp.tile([128, KO], F32, name=f"gb2c{rep}")
    lhs_ub = pp.tile([128, KO, 2], BF16, name=f"lhsub{rep}")
    lhs12 = pp.tile([128, KO, 2], F32R, name=f"lhs12{rep}")
    d_cr = pp.tile([128, KO], F32R, name=f"dcr{rep}")
    dv_c = pp.tile([128, KO], F32, name=f"dvc{rep}")

    ones_ctmp = pp.tile([128, KO], F32, name=f"onesct{rep}")
    nc.vector.memset(ones_ctmp[:], 1.0)
    nc.vector.memset(ones_cb[:], 1.0)
    nc.vector.tensor_copy(ones_cf[:], ones_ctmp[:])
    nc.vector.memset(ones_rb[:], 1.0)
    nc.vector.memset(ones_rf[:], 1.0)
    nc.vector.tensor_copy(ones_rr[:], ones_rf[:])
    eps_s = pp.tile([1, 1], F32, name=f"eps{rep}")
    nc.vector.memset(eps_s[:], EPS)

    smsa_c = cols[:, 0 * KO:1 * KO]
    shmsa_c = cols[:, 1 * KO:2 * KO]
    G_c = cols[:, 2 * KO:3 * KO]
    gm_c = cols[:, 3 * KO:4 * KO]
    gmlp_c = cols[:, 4 * KO:5 * KO]
    smlp_c = cols[:, 5 * KO:6 * KO]
    shmlp_c = cols[:, 6 * KO:7 * KO]

    # ============ scope A: mods + derived rows + col transposes ============
    with ExitStack() as sa:
        sb = sa.enter_context(tc.tile_pool(name=f"sa_sb{rep}", bufs=2))
        rw = sa.enter_context(tc.tile_pool(name=f"sa_rw{rep}", bufs=1))
        ps = sa.enter_context(tc.tile_pool(name=f"sa_ps{rep}", bufs=3, space="PSUM"))
        pst = sa.enter_context(tc.tile_pool(name=f"sa_pst{rep}", bufs=1, space="PSUM"))

        ccol_t = sb.tile([128, KO], F32, tag="ld", bufs=1)
        nc.sync.dma_start(ccol_t[:], io["c_col"].ap())
        silu_f = sb.tile([128, KO], F32, tag="ld2", bufs=1)
        nc.scalar.activation(silu_f[:], ccol_t[:], AF.Silu)
        nc.vector.tensor_copy(silu_bf[:], silu_f[:])

        mods = rw.tile([1, M6], F32)
        for j0, jn in _nsplits(M6, NCHUNK):
            wt = sb.tile([128, KO, NCHUNK], BF16, tag="wstream")
            nc.sync.dma_start(wt[:, :, :jn], wada_r[:, :, j0:j0 + jn])
            bt = sb.tile([1, NCHUNK], F32, tag="bstream")
            nc.sync.dma_start(bt[:, :jn], io["b_ada"].ap()[0:1, j0:j0 + jn])
            pt = ps.tile([1, NCHUNK], F32, tag="pss")
            for ko in range(KO):
                nc.tensor.matmul(pt[:, :jn], silu_bf[:, ko:ko + 1],
                                 wt[:, ko, :jn], start=(ko == 0),
                                 stop=(ko == KO - 1))
            nc.vector.tensor_add(mods[:, j0:j0 + jn], pt[:, :jn], bt[:, :jn])

        tap("mods", mods[:])
        nc.vector.tensor_scalar_add(smsa_row[:], mods[:, C:2 * C], 1.0)
        nc.vector.tensor_copy(shmsa_row[:], mods[:, 0:C])
        gm_row = rw.tile([1, C], F32)
        nc.vector.tensor_scalar_add(gm_row[:], mods[:, 2 * C:3 * C], 1.0)
        smlp_row = rw.tile([1, C], F32)
        nc.vector.tensor_scalar_add(smlp_row[:], mods[:, 4 * C:5 * C], 1.0)
        G_row = rw.tile([1, C], F32)
        nc.vector.tensor_mul(G_row[:], gm_row[:], smlp_row[:])

        rows1 = [smsa_row[:], shmsa_row[:], G_row[:], gm_row[:],
                 mods[:, 5 * C:6 * C], smlp_row[:], mods[:, 3 * C:4 * C]]
        tps = pst.tile([128, 7 * KO], F32)
        for r, rwap in enumerate(rows1):
            for ko in range(KO):
                nc.tensor.transpose(tps[:, r * KO + ko:r * KO + ko + 1],
                                    rwap[:, ko * 128:(ko + 1) * 128],
                                    ident[:1, :1])
        nc.vector.tensor_copy(cols[:], tps[:])
        bf2c_t = sb.tile([128, KO], F32, tag="ld", bufs=1)
        nc.sync.dma_start(bf2c_t[:], io["bf2c"].ap())
        nc.vector.tensor_mul(gb2_c[:], gmlp_c, bf2c_t[:])
        nc.vector.tensor_copy(lhs_ub[:, :, 0], smlp_c)
        nc.vector.tensor_copy(lhs_ub[:, :, 1], shmlp_c)

    # ============ scope B: x0 LN/modulate, kv, d/v, e/f/g ============
    with ExitStack() as sb_:
        sb = sb_.enter_context(tc.tile_pool(name=f"sb_sb{rep}", bufs=2))
        rw = sb_.enter_context(tc.tile_pool(name=f"sb_rw{rep}", bufs=1))
        ps = sb_.enter_context(tc.tile_pool(name=f"sb_ps{rep}", bufs=3, space="PSUM"))
        pst = sb_.enter_context(tc.tile_pool(name=f"sb_pst{rep}", bufs=1, space="PSUM"))

        x0t = sb.tile([128, KO, 2], F32R, tag="x0", bufs=1)
        nc.sync.dma_start(x0t[:], io["x0T"].ap())
        x0sq = sb.tile([128, KO, 2], F32R, tag="x0sq", bufs=1)
        nc.scalar.activation(x0sq[:], x0t[:], AF.Square)
        ps0a = ps.tile([2, NCHUNK], F32, tag="pss")
        for ko in range(KO):
            nc.tensor.matmul(ps0a[:1, 0:2], ones_cf[:, ko:ko + 1], x0t[:, ko, :],
                             start=(ko == 0), stop=(ko == KO - 1))
        ps0b = ps.tile([2, NCHUNK], F32, tag="pss")
        for ko in range(KO):
            nc.tensor.matmul(ps0b[:1, 0:2], ones_cf[:, ko:ko + 1], x0sq[:, ko, :],
                             start=(ko == 0), stop=(ko == KO - 1))
        st0 = rw.tile([1, 4], F32)
        nc.scalar.copy(st0[:, 0:2], ps0a[:1, 0:2])
        nc.scalar.copy(st0[:, 2:4], ps0b[:1, 0:2])
        mu0 = st0[:, 0:2]
        nc.vector.tensor_scalar_mul(mu0, mu0, 1.0 / C)        # mu
        nc.vector.tensor_scalar_mul(st0[:, 2:4], st0[:, 2:4], 1.0 / C)
        var0 = rw.tile([1, 2], F32)
        nc.vector.tensor_mul(var0[:], mu0, mu0)
        nc.vector.tensor_sub(var0[:], st0[:, 2:4], var0[:])
        nc.scalar.activation(var0[:], var0[:], AF.Sqrt, bias=eps_s[:1, :])
        rstd0 = rw.tile([1, 2], F32)
        nc.vector.reciprocal(rstd0[:], var0[:])
        pack = rw.tile([1, 4], F32)
        nc.vector.tensor_copy(pack[:, 0:2], mu0)
        nc.vector.tensor_copy(pack[:, 2:4], rstd0[:])
        psb = ps.tile([128, 4], F32, tag="psb")
        nc.tensor.matmul(psb[:], ones_rf[:], pack[:], start=True, stop=True)
        st0c = sb.tile([128, 4], F32, tag="st0c", bufs=1)
        nc.vector.tensor_copy(st0c[:], psb[:])

        xm0 = sb.tile([128, KO, 2], F32R, tag="xm0", bufs=1)
        for t in range(2):
            tmp0 = sb.tile([128, KO], F32, tag="tmp0")
            nc.vector.tensor_scalar(tmp0[:], x0t[:, :, t],
                                    st0c[:, t:t + 1], st0c[:, 2 + t:3 + t],
                                    op0=ALU.subtract, op1=ALU.mult)
            nc.vector.tensor_mul(tmp0[:], tmp0[:], smsa_c)
            nc.vector.tensor_add(xm0[:, :, t], tmp0[:], shmsa_c)

        kv_sb = rw.tile([2, 2 * C], F32)
        for j0, jn in _nsplits(2 * C, NCHUNK):
            wt = sb.tile([128, KO, NCHUNK], F32R, tag="wstreamr")
            nc.sync.dma_start(wt[:, :, :jn], w1_r[:, :, j0:j0 + jn])
            kvps = ps.tile([2, NCHUNK], F32, tag="pss")
            for ko in range(KO):
                nc.tensor.matmul(kvps[:, :jn], xm0[:, ko, :], wt[:, ko, :jn],
                                 start=(ko == 0), stop=(ko == KO - 1))
            bt = sb.tile([2, NCHUNK], F32, tag="bstream2")
            nc.sync.dma_start(bt[:, :jn], io["b1_2"].ap()[:, j0:j0 + jn])
            nc.vector.tensor_add(kv_sb[:, j0:j0 + jn], kvps[:, :jn], bt[:, :jn])

        tap("kv", kv_sb[:])
        tps2 = pst.tile([128, 4 * KO], F32, tag="tps2")
        for i in range(2 * KO):
            nc.tensor.transpose(tps2[:, 2 * i:2 * i + 2],
                                kv_sb[:, i * 128:(i + 1) * 128], ident[:2, :2])
        nc.vector.tensor_copy(kvT[:], tps2[:])
        d_c = rw.tile([128, KO], F32)
        nc.vector.tensor_sub(d_c[:], kvT[:, 0:KO, 0], kvT[:, 0:KO, 1])
        nc.vector.tensor_copy(d_cr[:], d_c[:])
        nc.vector.tensor_sub(dv_c[:], kvT[:, KO:2 * KO, 0], kvT[:, KO:2 * KO, 1])

        ea_row = rw.tile([1, C + 2], F32)
        for j0, jn in _nsplits(C + 2, NCHUNK):
            wt = sb.tile([128, KO, NCHUNK], F32R, tag="wstreamr")
            nc.sync.dma_start(wt[:, :, :jn], w2_r[:, :, j0:j0 + jn])
            pt = ps.tile([2, NCHUNK], F32, tag="pss")
            for ko in range(KO):
                nc.tensor.matmul(pt[:1, :jn], d_cr[:, ko:ko + 1], wt[:, ko, :jn],
                                 start=(ko == 0), stop=(ko == KO - 1))
            nc.scalar.copy(ea_row[:, j0:j0 + jn], pt[:1, :jn])
        tap("ea", ea_row[:])
        f_row = rw.tile([1, C], F32)
        nc.vector.tensor_mul(f_row[:], ea_row[:, :C], smsa_row[:])
        F1 = rw.tile([1, 1], F32)
        nc.vector.reduce_sum(F1[:], f_row[:], axis=AX.X)
        she_t = rw.tile([1, C], F32)
        nc.vector.tensor_mul(she_t[:], shmsa_row[:], ea_row[:, :C])
        g_sc = rw.tile([1, 1], F32)
        nc.vector.reduce_sum(g_sc[:], she_t[:], axis=AX.X)
        nc.vector.tensor_add(g_sc[:], g_sc[:], ea_row[:, C:C + 1])
        nc.vector.tensor_scalar_mul(gsc_s[:], g_sc[:], SCALE)

        tps3 = pst.tile([128, KO], F32, tag="tps3")
        for ko in range(KO):
            nc.tensor.transpose(tps3[:, ko:ko + 1],
                                f_row[:, ko * 128:(ko + 1) * 128], ident[:1, :1])
        nc.vector.tensor_copy(lhs12[:, :, 0], ones_cf[:])
        nc.vector.tensor_copy(lhs12[:, :, 1], tps3[:])
        F1_keep = pp.tile([1, 1], F32, name=f"F1k{rep}")
        nc.vector.tensor_copy(F1_keep[:], F1[:])

    # ============ scope C: x stats, p, Z build, resident load ============
    with ExitStack() as sc_:
        sb = sc_.enter_context(tc.tile_pool(name=f"sc_sb{rep}", bufs=1))
        rw = sc_.enter_context(tc.tile_pool(name=f"sc_rw{rep}", bufs=1))
        ps = sc_.enter_context(tc.tile_pool(name=f"sc_ps{rep}", bufs=3, space="PSUM"))
        psb_p = sc_.enter_context(tc.tile_pool(name=f"sc_psb{rep}", bufs=2, space="PSUM"))

        s1d = rw.tile([2, ntok], F32)
        s2r = rw.tile([1, ntok], F32)
        XC = 256
        for j in range(ntok // XC):
            xt = sb.tile([128, KO, XC], F32R, tag="xstream", bufs=2)
            nc.sync.dma_start(xt[:], xT_r[:, :, bass.ts(j, XC)])
            p1 = ps.tile([2, NCHUNK], F32, tag="pss")
            for ko in range(KO):
                nc.tensor.matmul(p1[:, :XC], lhs12[:, ko, :], xt[:, ko, :],
                                 start=(ko == 0), stop=(ko == KO - 1))
            nc.scalar.copy(s1d[:, bass.ts(j, XC)], p1[:, :XC])
            xsq = sb.tile([128, KO, XC], BF16, tag="xsq", bufs=2)
            nc.scalar.activation(xsq[:], xt[:], AF.Square)
            p2 = ps.tile([2, NCHUNK], F32, tag="pss")
            for ko in range(KO):
                nc.tensor.matmul(p2[:1, :XC], ones_cb[:, ko:ko + 1], xsq[:, ko, :],
                                 start=(ko == 0), stop=(ko == KO - 1))
            nc.scalar.copy(s2r[:, bass.ts(j, XC)], p2[:1, :XC])

        tap("s1d", s1d[:])
        tap("s2r", s2r[:])
        # p chain (in-place on rows; s1d row0 becomes mu, d_r becomes p)
        d_r = rw.tile([1, ntok], F32)
        nc.sync.dma_start(d_r[:], s1d[1:2, :])
        mu_r = s1d[0:1, :]
        nc.vector.tensor_scalar_mul(mu_r, mu_r, 1.0 / C)
        tmp_r = rw.tile([1, ntok], F32)
        nc.vector.tensor_mul(tmp_r[:], mu_r, mu_r)
        nc.vector.tensor_scalar_mul(s2r[:], s2r[:], 1.0 / C)
        nc.vector.tensor_sub(s2r[:], s2r[:], tmp_r[:])         # var
        nc.scalar.activation(s2r[:], s2r[:], AF.Sqrt, bias=eps_s[:1, :])
        nc.vector.reciprocal(s2r[:], s2r[:])                   # rstd
        nc.vector.tensor_scalar(tmp_r[:], mu_r, F1_keep[:1, :], None,
                                op0=ALU.mult)
        nc.vector.tensor_sub(d_r[:], d_r[:], tmp_r[:])         # D - mu*F1
        nc.vector.tensor_mul(d_r[:], d_r[:], s2r[:])
        p_row = d_r
        nc.scalar.activation(p_row[:1, :], d_r[:], AF.Sigmoid,
                             bias=gsc_s[:1, :], scale=SCALE)

        tap("p", p_row[:1, :])
        p_bc = sb.tile([128, ntok], F32, tag="pbc", bufs=1)
        for j in range(NB):
            pb = psb_p.tile([128, NCHUNK], F32, tag="psbc")
            nc.tensor.matmul(pb[:], ones_rf[:], p_row[:1, bass.ts(j, NCHUNK)],
                             start=True, stop=True)
            nc.vector.tensor_copy(p_bc[:, bass.ts(j, NCHUNK)], pb[:])
        tap("pbc", p_bc[:])
        for k in range(KO):
            z_t = sb.tile([128, ntok], BF16, tag="ztile", bufs=1)
            nc.vector.tensor_scalar(z_t[:], p_bc[:], dv_c[:, k:k + 1],
                                    kvT[:, KO + k, 1:2], op0=ALU.mult,
                                    op1=ALU.add)
            nc.sync.dma_start(flat_za[k * 128:(k + 1) * 128, :], z_t[:])
        tap("flat", flat_za[:, :].bitcast(mybir.dt.uint16))
        for k in range(KO):
            nc.sync.dma_start_transpose(y2T[:, k, :],
                                        flat_tok[:, k * 128:(k + 1) * 128])

    # late-persist pool (spans scope D + MLP loop)
    lt = persist.enter_context(tc.tile_pool(name=f"late{rep}", bufs=1))
    rb_bf = lt.tile([128, ntok], BF16, name=f"rb{rep}")             # 8 KB
    exrhs = lt.tile([3, ntok], BF16, name=f"exrhs{rep}")            # 12 KB
    ubb_bf = lt.tile([3, H], BF16, name=f"ubb{rep}")                # 13.8 KB
    nc.vector.memset(exrhs[:, :], 1.0)

    # ============ scope D: y2 stats, rstd2/extras, u/bb stream ============
    with ExitStack() as sd_:
        sb = sd_.enter_context(tc.tile_pool(name=f"sd_sb{rep}", bufs=2))
        rw = sd_.enter_context(tc.tile_pool(name=f"sd_rw{rep}", bufs=1))
        ps = sd_.enter_context(tc.tile_pool(name=f"sd_ps{rep}", bufs=3, space="PSUM"))
        psb_p = sd_.enter_context(tc.tile_pool(name=f"sd_psb{rep}", bufs=2, space="PSUM"))

        s1y = rw.tile([1, ntok], F32)
        s2y = rw.tile([1, ntok], F32)
        for j in range(NB):
            yg = sb.tile([128, KO, NCHUNK], F32R, tag="ygs", bufs=1)
            for ko in range(KO):
                nc.vector.tensor_scalar(yg[:, ko, :],
                                        y2T[:, ko, bass.ts(j, NCHUNK)],
                                        gm_c[:, ko:ko + 1], None, op0=ALU.mult)
            p1 = ps.tile([2, NCHUNK], F32, tag="pss")
            for ko in range(KO):
                nc.tensor.matmul(p1[:1, :], ones_cf[:, ko:ko + 1],
                                 yg[:, ko, :],
                                 start=(ko == 0), stop=(ko == KO - 1))
            nc.scalar.copy(s1y[:, bass.ts(j, NCHUNK)], p1[:1, :])
            nc.scalar.activation(yg[:], yg[:], AF.Square)
            p2 = ps.tile([2, NCHUNK], F32, tag="pss")
            for ko in range(KO):
                nc.tensor.matmul(p2[:1, :], ones_cf[:, ko:ko + 1], yg[:, ko, :],
                                 start=(ko == 0), stop=(ko == KO - 1))
            nc.scalar.copy(s2y[:, bass.ts(j, NCHUNK)], p2[:1, :])

        tap("s1y", s1y[:])
        tap("s2y", s2y[:])
        mu2_r = s1y[0:1, :]
        nc.vector.tensor_scalar_mul(mu2_r, mu2_r, 1.0 / C)
        tmp_r = rw.tile([1, ntok], F32)
        nc.vector.tensor_mul(tmp_r[:], mu2_r, mu2_r)
        nc.vector.tensor_scalar_mul(s2y[:], s2y[:], 1.0 / C)
        nc.vector.tensor_sub(s2y[:], s2y[:], tmp_r[:])
        nc.scalar.activation(s2y[:], s2y[:], AF.Sqrt, bias=eps_s[:1, :])
        nc.vector.reciprocal(s2y[:], s2y[:])                   # rstd2
        nc.vector.tensor_mul(tmp_r[:], mu2_r, s2y[:])
        nc.vector.tensor_scalar_mul(tmp_r[:], tmp_r[:], -1.0)  # -rstd2*mu2
        nc.vector.tensor_copy(exrhs[0:1, :], tmp_r[:])

        for j in range(NB):
            pb = psb_p.tile([128, NCHUNK], F32, tag="psbc")
            nc.tensor.matmul(pb[:], ones_rf[:], s2y[:, bass.ts(j, NCHUNK)],
                             start=True, stop=True)
            nc.vector.tensor_copy(rb_bf[:, bass.ts(j, NCHUNK)], pb[:])

        for j0, jn in _nsplits(H, NCHUNK):
            wt = sb.tile([128, KO, NCHUNK], BF16, tag="wstream")
            nc.sync.dma_start(wt[:, :, :jn], wf1_r[:, :, j0:j0 + jn])
            pt = ps.tile([2, NCHUNK], F32, tag="pss")
            for ko in range(KO):
                nc.tensor.matmul(pt[:, :jn], lhs_ub[:, ko, :], wt[:, ko, :jn],
                                 start=(ko == 0), stop=(ko == KO - 1))
            nc.scalar.copy(ubb_bf[0:2, j0:j0 + jn], pt[:, :jn])
        nc.sync.dma_start(ubb_bf[2:3, :], io["bf1b"].ap())

    tap("y2T", y2T[:, :, :])
    tap("exrhs", exrhs[:].bitcast(mybir.dt.uint16))
    tap("ubb", ubb_bf[:].bitcast(mybir.dt.uint16))
    tap("rb", rb_bf[:].bitcast(mybir.dt.uint16))
    # ================= MLP main loop =================
    with ExitStack() as sm:
        ya_p = sm.enter_context(tc.tile_pool(name=f"yaug{rep}", bufs=1))
        gl_p = sm.enter_context(tc.tile_pool(name=f"glp{rep}", bufs=1))
        w1_p = sm.enter_context(tc.tile_pool(name=f"wf1p{rep}", bufs=3))
        w2_p = sm.enter_context(tc.tile_pool(name=f"wf2p{rep}", bufs=2))
        ot_p = sm.enter_context(tc.tile_pool(name=f"otp{rep}", bufs=2))
        psA = sm.enter_context(tc.tile_pool(name=f"psA{rep}", bufs=2, space="PSUM"))
        psB = sm.enter_context(tc.tile_pool(name=f"psB{rep}", bufs=2, space="PSUM"))

        for j in range(NB):
            js = bass.ts(j, NCHUNK)
            yaug = ya_p.tile([128, KO, NCHUNK], BF16, tag="yaug")
            for ko in range(KO):
                nc.vector.tensor_scalar(yaug[:, ko, :], y2T[:, ko, js],
                                        G_c[:, ko:ko + 1], None, op0=ALU.mult)
                nc.vector.tensor_mul(yaug[:, ko, :], yaug[:, ko, :],
                                     rb_bf[:, js])
            gl = gl_p.tile([128, HO, NCHUNK], BF16, tag="gl")
            for m in range(HO):
                wt1 = w1_p.tile([128, KO, 128], BF16, tag="wf1t")
                nc.sync.dma_start(wt1[:], wf1_r[:, :, bass.ts(m, 128)])
                pa = psA.tile([128, NCHUNK], F32, tag="psa")
                for ko in range(KO):
                    nc.tensor.matmul(pa[:], wt1[:, ko, :], yaug[:, ko, :],
                                     start=(ko == 0), stop=False)
                nc.tensor.matmul(pa[:], ubb_bf[:, bass.ts(m, 128)],
                                 exrhs[:, js], start=False, stop=True)
                nc.scalar.activation(gl[:, m, :], pa[:], AF.Gelu_apprx_tanh)
            for m2 in range(KO):
                wt2 = w2_p.tile([128, HO, 128], BF16, tag="wf2t")
                nc.sync.dma_start(wt2[:], wf2_r[:, :, bass.ts(m2, 128)])
                pb = psB.tile([128, NCHUNK], F32, tag="psb2")
                for ho in range(HO):
                    nc.tensor.matmul(pb[:], wt2[:, ho, :], gl[:, ho, :],
                                     start=(ho == 0), stop=(ho == HO - 1))
                ot = ot_p.tile([128, NCHUNK], F32, tag="ot")
                nc.scalar.activation(ot[:], pb[:], AF.Copy,
                                     scale=gmlp_c[:, m2:m2 + 1])
                yr = ot_p.tile([128, NCHUNK], F32, tag="yr")
                nc.vector.tensor_scalar(yr[:], y2T[:, m2, js],
                                        gm_c[:, m2:m2 + 1],
                                        gb2_c[:, m2:m2 + 1],
                                        op0=ALU.mult, op1=ALU.add)
                nc.vector.tensor_add(ot[:], ot[:], yr[:])
                nc.sync.dma_start(io["outT"].ap()[bass.ts(m2, 128), js], ot[:])
    persist.close()


_prog_cache = {}


def _get_program(ntok, repeat=1):
    key = (ntok, repeat)
    if key not in _prog_cache:
        _prog_cache[key] = build_program(ntok, repeat)
    return _prog_cache[key]


def make_in_maps(inputs, ntok=4096):
    x = np.asarray(inputs["x"], np.float32)
    c = np.asarray(inputs["c"], np.float32)
    B = x.shape[0]
    bf = np.float16

    shared = {
        "w_ada": np.asarray(inputs["w_ada"], np.float32).astype(bf),
        "b_ada": np.ascontiguousarray(np.asarray(inputs["b_ada"], np.float32)[None, :]),
        "w1": np.asarray(inputs["w1"], np.float32),
        "b1_2": np.ascontiguousarray(
            np.tile(np.asarray(inputs["b1"], np.float32)[None, :], (2, 1))),
        "w2Ta": np.ascontiguousarray(np.concatenate(
            [np.asarray(inputs["w2"], np.float32).T,
             np.asarray(inputs["b2"], np.float32)[:, None],
             np.zeros((C, 1), np.float32)], axis=1)),
        "wf1": np.asarray(inputs["w_fc1"], np.float32).astype(bf),
        "bf1b": np.ascontiguousarray(
            np.asarray(inputs["b_fc1"], np.float32)[None, :]).astype(bf),
        "wf2": np.asarray(inputs["w_fc2"], np.float32).astype(bf),
        "bf2c": np.ascontiguousarray(
            np.asarray(inputs["b_fc2"], np.float32).reshape(KO, 128).T),
    }
    in_maps = []
    for b in range(B):
        xb = x[b, :ntok, :]
        m = {
            "xT": np.ascontiguousarray(xb.T),
            "x0T": np.ascontiguousarray(
                xb[:2, :].T.reshape(KO, 128, 2).transpose(1, 0, 2)),
            "c_col": np.ascontiguousarray(c[b].reshape(KO, 128).T),
        }
        m.update(shared)
        in_maps.append(m)
    return in_maps


def kernel(**inputs):
    x = np.asarray(inputs["x"], np.float32)
    B, ntok, _ = x.shape
    nc = _get_program(ntok)
    in_maps = make_in_maps(inputs, ntok)
    res = run_bass_kernel_spmd(nc, in_maps, list(range(len(in_maps))))
    out = np.stack([np.asarray(res.results[b]["outT"], np.float32).T
                    for b in range(B)])
    return np.ascontiguousarray(out)

